# revision 5
# baseline (speedup 1.0000x reference)
"""Trainium2 Bass kernel for nn_Attention_layer (per-label MLP attention).

Computes, for full inputs:
    h = relu(cat(label_emb, unlabel_emb) @ W1 + b1)        [N, B, H]
    scores = h @ W2 + b2                                   [N, B]
    out = softmax(scores.T * dis_lab, axis=1)              [B, N]

Distribution: pure data-parallel over batch B across 8 NeuronCores
(B=1024 -> 128 rows/core). No collectives needed; softmax is over the
station axis N which stays local.

Key device-side structure (per core):
  - label_emb shard is cast to bf16 on host and DMA-transpose-loaded as
    [EMB, b] tiles (hardware xbar transpose; needs 2-byte dtype).
  - W2 is folded into W1 on host (W1' = W1 * w2 per column) and columns
    are sorted so positive-w2 columns come first. Then
       scores = sum_pos relu(h') + sum_neg min(h', 0)
    which maps onto relu/min reductions along the free axis.
  - Main matmul h'[b, Hcols] runs on PE in bf16 (full rate), PSUM f32.
  - unlabel contribution (shared across stations) is accumulated into
    PSUM via an identity matmul (PE) - cheaper than a vector add.
  - relu + signed sum are fused: ACT activation(Relu, accum_out=...) or
    DVE tensor_tensor_reduce(max/min 0, reduce add), split across both
    engines to balance load. No h' materialization in SBUF at all.
  - Tail: scores * dis_lab, numerically-stable softmax over 64 cols.
"""

import os
import sys

for _p in (
    "/root/.axon_site",
    "/root/.axon_site/_ro/trn_rl_repo",
    "/root/.axon_site/_ro/pypackages",
):
    if _p not in sys.path and os.path.isdir(_p):
        sys.path.append(_p)

import ml_dtypes
import numpy as np

import concourse.bass as bass
import concourse.mybir as mybir
import concourse.tile as tile
from concourse import bacc
from concourse.bass_utils import run_bass_kernel_spmd
from concourse.masks import make_identity

N, B, EMB, UEMB, H = 64, 1024, 256, 256, 1024
N_CORES = 8
BS = B // N_CORES  # 128 batch rows per core
KL = EMB // 128  # label-emb contraction chunks
KU = UEMB // 128  # unlabel-emb contraction chunks
F32 = mybir.dt.float32
BF16 = mybir.dt.bfloat16

# Tuning knobs: which station tiles run their relu+reduce on DVE (vs ACT),
# and which run their unlabel-add on DVE (vs PE identity-matmul).
RELU_ON_DVE = lambda n: (n % 8) in (3, 7)  # noqa: E731
ADD_ON_DVE = lambda n: False  # noqa: E731
USE_DMA_T = os.environ.get("NO_DMA_T", "") == ""  # label-emb transpose via DMA xbar

PROFILE = False
LAST_EXEC_NS = None
TRACE_DIR = None

_cache = {}


def _build(jpos, b2val):
    nc = bacc.Bacc("TRN2", target_bir_lowering=False, debug=False,
                   num_devices=N_CORES)
    xlab = nc.dram_tensor("xlab", [N, BS, EMB], BF16, kind="ExternalInput").ap()
    xunl = nc.dram_tensor("xunl", [BS, UEMB], BF16, kind="ExternalInput").ap()
    dis = nc.dram_tensor("dis", [BS, N], F32, kind="ExternalInput").ap()
    w1p = nc.dram_tensor("w1p", [EMB + UEMB, H], BF16, kind="ExternalInput").ap()
    b1p = nc.dram_tensor("b1p", [H], F32, kind="ExternalInput").ap()
    out = nc.dram_tensor("out", [BS, N], F32, kind="ExternalOutput").ap()

    with tile.TileContext(nc) as tc:
        _emit(tc, out, xlab, xunl, dis, w1p, b1p, jpos, b2val)
    nc.compile()
    return nc


def _emit(tc, out, xlab, xunl, dis, w1p, b1p, jpos, b2val):
    nc = tc.nc
    AF = mybir.ActivationFunctionType
    ALU = mybir.AluOpType

    with tc.tile_pool(name="consts", bufs=1) as consts:
        # --- constants / weights ---
        ident = consts.tile([128, 128], BF16, tag="ident")
        make_identity(nc, ident)

        w1sb = []
        for k in range(KL + KU):
            t = consts.tile([128, H], BF16, tag=f"w1_{k}")
            nc.sync.dma_start(out=t, in_=w1p[128 * k:128 * (k + 1), :])
            w1sb.append(t)

        dis_sb = consts.tile([128, N], F32, tag="dis")
        nc.sync.dma_start(out=dis_sb, in_=dis)

        b1bc = consts.tile([128, H], F32, tag="b1bc")
        b1_bcast = bass.AP(tensor=b1p.tensor, offset=b1p.offset,
                           ap=[[0, 128]] + list(b1p.ap))
        nc.sync.dma_start(out=b1bc, in_=b1_bcast)

        # --- unlabel branch: unl_h = xunl @ W1_unl' + b1'  (bf16, shared) ---
        unl_sb = consts.tile([128, H], BF16, tag="unl")
        with tc.tile_pool(name="pre_psum", bufs=1, space="PSUM") as pre_psum:
            xunlT = []
            for k in range(KU):
                tsb = consts.tile([128, 128], BF16, tag=f"xunlT_{k}")
                nc.sync.dma_start_transpose(
                    out=tsb, in_=xunl[:, 128 * k:128 * (k + 1)])
                xunlT.append(tsb)
            psu = pre_psum.tile([128, H], F32, tag="psu")
            for half in range(2):
                hs = slice(512 * half, 512 * (half + 1))
                for k in range(KU):
                    nc.tensor.matmul(psu[:, hs], xunlT[k], w1sb[KL + k][:, hs],
                                     start=(k == 0), stop=(k == KU - 1))
            # unl_sb = bf16(psu + b1bc)
            nc.vector.tensor_tensor(out=unl_sb, in0=psu, in1=b1bc, op=ALU.add)

        # --- prefetch all transposed label tiles: xlabT[:, n, k, :] ---
        xlabT = consts.tile([128, N, KL, 128], BF16, tag="xlabT")
        if USE_DMA_T:
            for n in range(N):
                for k in range(KL):
                    nc.sync.dma_start_transpose(
                        out=xlabT[:, n, k, :],
                        in_=xlab[n, :, 128 * k:128 * (k + 1)])
        else:
            with tc.tile_pool(name="tr_psum", bufs=4, space="PSUM") as trp, \
                 tc.tile_pool(name="tr_sbuf", bufs=4) as trs:
                for n in range(N):
                    for k in range(KL):
                        nat = trs.tile([128, 128], BF16, tag="nat")
                        nc.sync.dma_start(
                            out=nat, in_=xlab[n, :, 128 * k:128 * (k + 1)])
                        tp = trp.tile([128, 128], BF16, tag="tp")
                        nc.tensor.transpose(tp, nat, ident)
                        nc.vector.tensor_copy(xlabT[:, n, k, :], tp)

        # --- score accumulators ---
        sAp = consts.tile([128, N], F32, tag="sAp")
        sAm = consts.tile([128, N], F32, tag="sAm")
        sDp = consts.tile([128, N], F32, tag="sDp")
        sDm = consts.tile([128, N], F32, tag="sDm")
        for t in (sAp, sAm, sDp, sDm):
            nc.gpsimd.memset(t, 0.0)

        # --- main loop over stations ---
        with tc.tile_pool(name="psum", bufs=3, space="PSUM") as psum_pool:
            for n in range(N):
                ph = psum_pool.tile([128, H], F32, tag="ph")
                add_pe = not ADD_ON_DVE(n)
                for half in range(2):
                    hs = slice(512 * half, 512 * (half + 1))
                    nc.tensor.matmul(ph[:, hs], xlabT[:, n, 0, :],
                                     w1sb[0][:, hs], start=True, stop=False)
                    nc.tensor.matmul(ph[:, hs], xlabT[:, n, 1, :],
                                     w1sb[1][:, hs], start=False,
                                     stop=not add_pe)
                    if add_pe:
                        nc.tensor.matmul(ph[:, hs], ident, unl_sb[:, hs],
                                         start=False, stop=True)
                if not add_pe:
                    nc.vector.tensor_tensor(out=ph, in0=ph, in1=unl_sb,
                                            op=ALU.add)
                if RELU_ON_DVE(n):
                    nc.vector.tensor_scalar(
                        out=ph[:, :jpos], in0=ph[:, :jpos], scalar1=0.0,
                        scalar2=None, op0=ALU.max, op1=ALU.add,
                        accum_out=sDp[:, n:n + 1])
                    nc.vector.tensor_scalar(
                        out=ph[:, jpos:], in0=ph[:, jpos:], scalar1=0.0,
                        scalar2=None, op0=ALU.min, op1=ALU.add,
                        accum_out=sDm[:, n:n + 1])
                else:
                    nc.scalar.activation(
                        out=ph[:, :jpos], in_=ph[:, :jpos], func=AF.Relu,
                        accum_out=sAp[:, n:n + 1])
                    # relu(-x) summed; subtracted at assembly = sum min(x,0)
                    nc.scalar.activation(
                        out=ph[:, jpos:], in_=ph[:, jpos:], func=AF.Relu,
                        scale=-1.0, accum_out=sAm[:, n:n + 1])

        # --- scores assembly + softmax tail (all [128, N] sized) ---
        t1 = consts.tile([128, N], F32, tag="t1")
        t2 = consts.tile([128, N], F32, tag="t2")
        nc.vector.tensor_tensor(out=t1, in0=sAp, in1=sAm, op=ALU.subtract)
        nc.vector.tensor_tensor(out=t2, in0=sDp, in1=sDm, op=ALU.add)
        nc.vector.tensor_tensor(out=t1, in0=t1, in1=t2, op=ALU.add)
        if b2val != 0.0:
            nc.vector.tensor_scalar_add(t1, t1, float(b2val))
        att = consts.tile([128, N], F32, tag="att")
        nc.vector.tensor_tensor(out=att, in0=t1, in1=dis_sb, op=ALU.mult)

        mx = consts.tile([128, 1], F32, tag="mx")
        nc.vector.reduce_max(mx, att, axis=mybir.AxisListType.X)
        mxn = consts.tile([128, 1], F32, tag="mxn")
        nc.vector.tensor_scalar_mul(mxn, mx, -1.0)
        ex = consts.tile([128, N], F32, tag="ex")
        sume = consts.tile([128, 1], F32, tag="sume")
        nc.scalar.activation(out=ex, in_=att, func=AF.Exp, bias=mxn,
                             scale=1.0, accum_out=sume)
        rs = consts.tile([128, 1], F32, tag="rs")
        nc.vector.reciprocal(rs, sume)
        res = consts.tile([128, N], F32, tag="res")
        nc.vector.tensor_scalar_mul(res, ex, rs)
        nc.sync.dma_start(out=out, in_=res)


def kernel(unlabel_emb, label_emb, dis_lab, W1, b1, W2, b2):
    global LAST_EXEC_NS, TRACE_DIR
    unlabel_emb = np.asarray(unlabel_emb, dtype=np.float32)
    label_emb = np.asarray(label_emb, dtype=np.float32)
    dis_lab = np.asarray(dis_lab, dtype=np.float32)
    W1 = np.asarray(W1, dtype=np.float32)
    b1 = np.asarray(b1, dtype=np.float32)
    W2 = np.asarray(W2, dtype=np.float32)
    b2 = np.asarray(b2, dtype=np.float32)

    # Fold W2 into W1 columns; sort columns so positive-w2 ones come first.
    w2 = W2[:, 0]
    pos = w2 > 0
    perm = np.argsort(~pos, kind="stable")
    jpos = int(pos.sum())
    W1f = (W1 * w2[None, :])[:, perm]
    b1f = (b1 * w2)[perm]
    b2val = float(b2[0])

    key = (jpos, b2val)
    if key not in _cache:
        _cache[key] = _build(jpos, b2val)
    nc = _cache[key]

    w1p_np = W1f.astype(ml_dtypes.bfloat16)
    b1p_np = b1f.astype(np.float32)
    in_maps = []
    for c in range(N_CORES):
        sh = slice(c * BS, (c + 1) * BS)
        in_maps.append({
            "xlab": np.ascontiguousarray(label_emb[:, sh, :]).astype(ml_dtypes.bfloat16),
            "xunl": unlabel_emb[sh].astype(ml_dtypes.bfloat16),
            "dis": np.ascontiguousarray(dis_lab[sh]),
            "w1p": w1p_np,
            "b1p": b1p_np,
        })

    kwargs = {}
    if PROFILE:
        import ntff_shim  # noqa: F401  (registers the axon NTFF hook)
        import tempfile
        TRACE_DIR = tempfile.mkdtemp(prefix="bass_trace_")
        kwargs = dict(trace=True, tmpdir=TRACE_DIR)
    res = run_bass_kernel_spmd(nc, in_maps, core_ids=list(range(N_CORES)),
                               **kwargs)
    if PROFILE:
        LAST_EXEC_NS = res.exec_time_ns
    return np.concatenate([res.results[c]["out"] for c in range(N_CORES)],
                          axis=0)


# revision 6
# speedup vs baseline: 1.5443x; 1.5443x over previous
"""Trainium2 Bass kernel for nn_Attention_layer (per-label MLP attention).

Computes, for full inputs:
    h = relu(cat(label_emb, unlabel_emb) @ W1 + b1)        [N, B, H]
    scores = h @ W2 + b2                                   [N, B]
    out = softmax(scores.T * dis_lab, axis=1)              [B, N]

Distribution: pure data-parallel over batch B across 8 NeuronCores
(B=1024 -> 128 rows/core). No collectives; softmax is over the station
axis N which stays local to a core.

Host prep: W2 is folded into W1 (W1' = W1 * w2 per column), columns
sorted so positive-w2 columns come first; then
    scores = sum_pos relu(h') + sum_neg min(h', 0)
Activations/weights are cast to bf16 and label/unlabel embeddings are
pre-transposed to [K, batch] layout so all device DMAs are contiguous.

Device (per core, per station n):
  PE:  psum[128b, 1024] = I @ unl_h' (start) + xlabT_k @ W1'_k   (bf16)
       (the identity matmul injects the shared unlabel contribution -
        an exact rank-128 K-extension - cheaper than any vector add)
  relu+signed-sum per station, balanced across engines, 3 modes:
    1: ACT activation(Relu, accum_out) on [:jpos] / scale=-1 on [jpos:]
    2: ACT relu psum->sbuf bf16, then DVE reduce_sum x2
    3: DVE tensor_scalar(max/min 0, op1=add, accum_out) x2
  Tail: scores * dis_lab, stable softmax over the 64 stations.
"""

import os
import sys

for _p in (
    "/root/.axon_site",
    "/root/.axon_site/_ro/trn_rl_repo",
    "/root/.axon_site/_ro/pypackages",
):
    if _p not in sys.path and os.path.isdir(_p):
        sys.path.append(_p)

import ml_dtypes
import numpy as np

import concourse.bass as bass
import concourse.mybir as mybir
import concourse.tile as tile
from concourse import bacc
from concourse.bass_utils import run_bass_kernel_spmd
from concourse.masks import make_identity

N, B, EMB, UEMB, H = 64, 1024, 256, 256, 1024
N_CORES = 8
BS = B // N_CORES  # 128 batch rows per core
KL = EMB // 128  # label-emb contraction chunks
KU = UEMB // 128  # unlabel-emb contraction chunks
F32 = mybir.dt.float32
BF16 = mybir.dt.bfloat16

# Tuning knobs.
_RELU_MODE_PAT = [1, 2, 2, 3, 1, 2, 2, 3]
RELU_MODE = lambda n: _RELU_MODE_PAT[n % len(_RELU_MODE_PAT)]  # noqa: E731
ADD_ON_DVE = lambda n: False  # noqa: E731

PROFILE = False
LAST_EXEC_NS = None
TRACE_DIR = None

_cache = {}


def _build(jpos, b2val):
    nc = bacc.Bacc("TRN2", target_bir_lowering=False, debug=False,
                   num_devices=N_CORES)
    xlabT = nc.dram_tensor("xlabT", [KL, 128, N, BS], BF16,
                           kind="ExternalInput").ap()
    xunlT = nc.dram_tensor("xunlT", [KU, 128, BS], BF16,
                           kind="ExternalInput").ap()
    dis = nc.dram_tensor("dis", [BS, N], F32, kind="ExternalInput").ap()
    w1p = nc.dram_tensor("w1p", [EMB + UEMB, H], BF16,
                         kind="ExternalInput").ap()
    b1p = nc.dram_tensor("b1p", [H], F32, kind="ExternalInput").ap()
    out = nc.dram_tensor("out", [BS, N], F32, kind="ExternalOutput").ap()

    with tile.TileContext(nc) as tc:
        _emit(tc, out, xlabT, xunlT, dis, w1p, b1p, jpos, b2val)
    nc.compile()
    return nc


def _emit(tc, out, xlabT_d, xunlT_d, dis, w1p, b1p, jpos, b2val):
    nc = tc.nc
    AF = mybir.ActivationFunctionType
    ALU = mybir.AluOpType

    with tc.tile_pool(name="consts", bufs=1) as consts:
        # --- constants / weights ---
        ident = consts.tile([128, 128], BF16, tag="ident")
        make_identity(nc, ident)

        w1sb = []
        for k in range(KL + KU):
            t = consts.tile([128, H], BF16, tag=f"w1_{k}")
            nc.sync.dma_start(out=t, in_=w1p[128 * k:128 * (k + 1), :])
            w1sb.append(t)

        dis_sb = consts.tile([128, N], F32, tag="dis")
        nc.sync.dma_start(out=dis_sb, in_=dis)

        b1bc = consts.tile([128, H], F32, tag="b1bc")
        b1_bcast = bass.AP(tensor=b1p.tensor, offset=b1p.offset,
                           ap=[[0, 128]] + list(b1p.ap))
        nc.sync.dma_start(out=b1bc, in_=b1_bcast)

        # --- transposed label embeddings: two big contiguous DMAs ---
        xlabT = consts.tile([128, KL, N, 128], BF16, tag="xlabT")
        for k in range(KL):
            nc.sync.dma_start(out=xlabT[:, k, :, :], in_=xlabT_d[k])

        # --- unlabel branch: unl_h' = xunlT.T @ W1_unl' + b1' (bf16) ---
        unl_sb = consts.tile([128, H], BF16, tag="unl")
        xunlT = []
        for k in range(KU):
            t = consts.tile([128, 128], BF16, tag=f"xunlT_{k}")
            nc.sync.dma_start(out=t, in_=xunlT_d[k])
            xunlT.append(t)
        with tc.tile_pool(name="pre_psum", bufs=1, space="PSUM") as pre_psum:
            psu = pre_psum.tile([128, H], F32, tag="psu")
            for half in range(2):
                hs = slice(512 * half, 512 * (half + 1))
                for k in range(KU):
                    nc.tensor.matmul(psu[:, hs], xunlT[k], w1sb[KL + k][:, hs],
                                     start=(k == 0), stop=(k == KU - 1))
            nc.vector.tensor_tensor(out=unl_sb, in0=psu, in1=b1bc, op=ALU.add)

        # --- score accumulators (per engine path, pos/neg ranges) ---
        sAp = consts.tile([128, N], F32, tag="sAp")
        sAm = consts.tile([128, N], F32, tag="sAm")
        sDp = consts.tile([128, N], F32, tag="sDp")
        sDm = consts.tile([128, N], F32, tag="sDm")
        for t in (sAp, sAm, sDp, sDm):
            nc.gpsimd.memset(t, 0.0)

        # --- main loop over stations ---
        with tc.tile_pool(name="psum", bufs=3, space="PSUM") as psum_pool, \
             tc.tile_pool(name="relu_sb", bufs=3) as relu_pool:
            for n in range(N):
                ph = psum_pool.tile([128, H], F32, tag="ph")
                add_pe = not ADD_ON_DVE(n)
                if add_pe:
                    # inject shared unlabel term first (starts accumulation),
                    # so the identity weights serve 2 matmuls per load
                    for half in range(2):
                        hs = slice(512 * half, 512 * (half + 1))
                        nc.tensor.matmul(ph[:, hs], ident, unl_sb[:, hs],
                                         start=True, stop=False)
                for k in range(KL):
                    for half in range(2):
                        hs = slice(512 * half, 512 * (half + 1))
                        nc.tensor.matmul(ph[:, hs], xlabT[:, k, n, :],
                                         w1sb[k][:, hs],
                                         start=(k == 0 and not add_pe),
                                         stop=(k == KL - 1))
                if not add_pe:
                    nc.vector.tensor_tensor(out=ph, in0=ph, in1=unl_sb,
                                            op=ALU.add)
                mode = RELU_MODE(n)
                if mode == 1:
                    nc.scalar.activation(
                        out=ph[:, :jpos], in_=ph[:, :jpos], func=AF.Relu,
                        accum_out=sAp[:, n:n + 1])
                    # relu(-x) summed; subtracted at assembly = sum min(x,0)
                    nc.scalar.activation(
                        out=ph[:, jpos:], in_=ph[:, jpos:], func=AF.Relu,
                        scale=-1.0, accum_out=sAm[:, n:n + 1])
                elif mode == 2:
                    rl = relu_pool.tile([128, H], BF16, tag="rl")
                    nc.scalar.activation(out=rl[:, :jpos], in_=ph[:, :jpos],
                                         func=AF.Relu)
                    nc.scalar.activation(out=rl[:, jpos:], in_=ph[:, jpos:],
                                         func=AF.Relu, scale=-1.0)
                    nc.vector.reduce_sum(sDp[:, n:n + 1], rl[:, :jpos],
                                         axis=mybir.AxisListType.X)
                    nc.vector.reduce_sum(sDm[:, n:n + 1], rl[:, jpos:],
                                         axis=mybir.AxisListType.X)
                else:
                    nc.vector.tensor_scalar(
                        out=ph[:, :jpos], in0=ph[:, :jpos], scalar1=0.0,
                        scalar2=None, op0=ALU.max, op1=ALU.add,
                        accum_out=sDp[:, n:n + 1])
                    nc.vector.tensor_scalar(
                        out=ph[:, jpos:], in0=ph[:, jpos:], scalar1=0.0,
                        scalar2=None, op0=ALU.min, op1=ALU.add,
                        accum_out=sDm[:, n:n + 1])

        # --- scores assembly + softmax tail (all [128, N] sized) ---
        # station n used either the A path (sAp - sAm) or D path:
        #   mode 3: sDp + sDm (min-sums are negative)
        #   mode 2: sDp - sDm (reduced relu(-x) sums are positive)
        # Build via sign vector applied to sDm.
        sgn = consts.tile([128, N], F32, tag="sgn")
        for n in range(N):
            v = 1.0 if RELU_MODE(n) == 3 else -1.0
            nc.gpsimd.memset(sgn[:, n:n + 1], v)
        t1 = consts.tile([128, N], F32, tag="t1")
        t2 = consts.tile([128, N], F32, tag="t2")
        nc.vector.tensor_tensor(out=t1, in0=sAp, in1=sAm, op=ALU.subtract)
        nc.vector.tensor_tensor(out=t2, in0=sDm, in1=sgn, op=ALU.mult)
        nc.vector.tensor_tensor(out=t2, in0=t2, in1=sDp, op=ALU.add)
        nc.vector.tensor_tensor(out=t1, in0=t1, in1=t2, op=ALU.add)
        if b2val != 0.0:
            nc.vector.tensor_scalar_add(t1, t1, float(b2val))
        att = consts.tile([128, N], F32, tag="att")
        nc.vector.tensor_tensor(out=att, in0=t1, in1=dis_sb, op=ALU.mult)

        mx = consts.tile([128, 1], F32, tag="mx")
        nc.vector.reduce_max(mx, att, axis=mybir.AxisListType.X)
        mxn = consts.tile([128, 1], F32, tag="mxn")
        nc.vector.tensor_scalar_mul(mxn, mx, -1.0)
        ex = consts.tile([128, N], F32, tag="ex")
        sume = consts.tile([128, 1], F32, tag="sume")
        nc.scalar.activation(out=ex, in_=att, func=AF.Exp, bias=mxn,
                             scale=1.0, accum_out=sume)
        rs = consts.tile([128, 1], F32, tag="rs")
        nc.vector.reciprocal(rs, sume)
        res = consts.tile([128, N], F32, tag="res")
        nc.vector.tensor_scalar_mul(res, ex, rs)
        nc.sync.dma_start(out=out, in_=res)


def kernel(unlabel_emb, label_emb, dis_lab, W1, b1, W2, b2):
    global LAST_EXEC_NS, TRACE_DIR
    unlabel_emb = np.asarray(unlabel_emb, dtype=np.float32)
    label_emb = np.asarray(label_emb, dtype=np.float32)
    dis_lab = np.asarray(dis_lab, dtype=np.float32)
    W1 = np.asarray(W1, dtype=np.float32)
    b1 = np.asarray(b1, dtype=np.float32)
    W2 = np.asarray(W2, dtype=np.float32)
    b2 = np.asarray(b2, dtype=np.float32)

    # Fold W2 into W1 columns; sort columns so positive-w2 ones come first.
    w2 = W2[:, 0]
    pos = w2 > 0
    perm = np.argsort(~pos, kind="stable")
    jpos = int(pos.sum())
    W1f = (W1 * w2[None, :])[:, perm]
    b1f = (b1 * w2)[perm]
    b2val = float(b2[0])

    key = (jpos, b2val)
    if key not in _cache:
        _cache[key] = _build(jpos, b2val)
    nc = _cache[key]

    w1p_np = W1f.astype(ml_dtypes.bfloat16)
    b1p_np = b1f.astype(np.float32)
    in_maps = []
    for c in range(N_CORES):
        sh = slice(c * BS, (c + 1) * BS)
        # [N, BS, EMB] -> [EMB, N, BS] -> [KL, 128, N, BS]
        lab_t = np.ascontiguousarray(
            label_emb[:, sh, :].transpose(2, 0, 1)).reshape(KL, 128, N, BS)
        unl_t = np.ascontiguousarray(
            unlabel_emb[sh].T).reshape(KU, 128, BS)
        in_maps.append({
            "xlabT": lab_t.astype(ml_dtypes.bfloat16),
            "xunlT": unl_t.astype(ml_dtypes.bfloat16),
            "dis": np.ascontiguousarray(dis_lab[sh]),
            "w1p": w1p_np,
            "b1p": b1p_np,
        })

    kwargs = {}
    if PROFILE:
        import ntff_shim  # noqa: F401  (registers the axon NTFF hook)
        import tempfile
        TRACE_DIR = tempfile.mkdtemp(prefix="bass_trace_")
        kwargs = dict(trace=True, tmpdir=TRACE_DIR)
    res = run_bass_kernel_spmd(nc, in_maps, core_ids=list(range(N_CORES)),
                               **kwargs)
    if PROFILE:
        LAST_EXEC_NS = res.exec_time_ns
    return np.concatenate([res.results[c]["out"] for c in range(N_CORES)],
                          axis=0)
